# revision 13
# baseline (speedup 1.0000x reference)
"""Trainium2 Bass kernel for an eager bidirectional attention block.

Reference computation (per batch b):
    q,k,v = (x @ Wq + bq), (x @ Wk + bk), (x @ Wv + bv)   split into 16 heads of 64
    scores = q @ k^T / sqrt(dh);  scores[:, masked_k] = -inf
    out = softmax(scores) @ v;    y = concat_heads(out) @ Wo + bo

Sharding (8 cores): core c owns batch b = c//2 and heads [8*(c%2), 8*(c%2)+8).
Each core computes a partial y_c = sum_{its heads} softmax(..) v_h @ Wo[h-rows, :]
(fp32, [S, D]); the host sums the two partials per batch and adds bo.
No collectives are needed.

Per-core layout strategy (all matmuls bf16, fp32 PSUM accumulate):
  - x is uploaded transposed (x^T [D, S]) so QKV projections produce
    Q^T/K^T [dh, S] directly (2 heads packed per 128-partition tile).
  - scores are computed transposed: S^T [k, q] = (K^T-tile)^T-stationary @ Q^T.
  - mask is folded into V:  V' = [V * m | m]  (65 columns per head); the PV
    matmul O^T = V'^T @ exp(S^T) then yields both the unnormalized output
    (rows 0..63) and the softmax denominator (row 64) in one accumulation.
  - 1/sqrt(dh) is folded into Wq/bq on the host.
  - normalization: reciprocal of row 64, partition-broadcast, multiply.
  - final projection contracts the stacked normalized O^T against Wo rows.

Scheduling (v2): attention runs in q-half passes so the score PSUM tile
([128,1024], 2 banks) double-buffers against the PV accumulator ([65,1024],
2 banks) and one shared [128,512] projection pool — 8 banks total. Emission
order V-proj, QK(g0), then attention pairs with the next group's QK
projections interleaved, keeping ACT (the exp engine, the critical path)
continuously fed.
"""

import numpy as np
import ml_dtypes

BF16 = ml_dtypes.bfloat16

# Full problem dims
B, S, D, H, DH = 4, 2048, 1024, 16, 64
N_CORES = 8
HC = 8  # heads per core


def build_nc(S=S, D=D, HC=HC, dh=DH, num_devices=N_CORES, reps=1, probe=None):
    from concourse import bacc
    import concourse.mybir as mybir
    from concourse.tile import TileContext

    f32 = mybir.dt.float32
    bf16 = mybir.dt.bfloat16
    Exp = mybir.ActivationFunctionType.Exp
    Mult = mybir.AluOpType.mult

    G = HC // 2           # 2-head groups
    DT = D // 128         # contraction tiles for projections
    KT = S // 128         # key-position tiles
    CH = min(512, S)      # psum chunk width (one bank)
    HD = HC * dh          # total head dim per core (512)
    WOT = HD // 128       # Wo k-tiles (= G)
    DCH = D // CH         # output chunks in final proj
    QHW = min(1024, S)    # attention q-pass width (2 banks)
    NP = S // QHW         # q passes per head
    SC = QHW // CH        # score-chunks per pass

    nc = bacc.Bacc("TRN2", target_bir_lowering=False, debug=False,
                   num_devices=num_devices)

    xT_d = nc.dram_tensor("xT", [D, S], bf16, kind="ExternalInput").ap()
    wq_d = nc.dram_tensor("wq", [D, HD], bf16, kind="ExternalInput").ap()
    wk_d = nc.dram_tensor("wk", [D, HD], bf16, kind="ExternalInput").ap()
    wv_d = nc.dram_tensor("wv", [D, HD], bf16, kind="ExternalInput").ap()
    wo_d = nc.dram_tensor("wo", [HD, D], bf16, kind="ExternalInput").ap()
    mkf_d = nc.dram_tensor("mkf", [128, KT], f32, kind="ExternalInput").ap()
    bq_d = nc.dram_tensor("bqc", [128, G], f32, kind="ExternalInput").ap()
    bk_d = nc.dram_tensor("bkc", [128, G], f32, kind="ExternalInput").ap()
    bv_d = nc.dram_tensor("bvr", [1, HD], bf16, kind="ExternalInput").ap()
    y_d = nc.dram_tensor("y", [S, D], f32, kind="ExternalOutput").ap()

    with TileContext(nc) as tc:
      for _rep in range(reps):
        with tc.tile_pool(name="const", bufs=1) as cp:
            # Small, critical-path transfers first: the mask (gates V'
            # prep), biases, and the Q/K/V weights; x^T is split into
            # column chunks so the first projection chunk only waits on
            # the leading slice of every dt-tile; Wo last (needed at the
            # very end only).
            mkf = cp.tile([128, KT], f32, tag="mkf")
            nc.sync.dma_start(mkf[:], mkf_d[:, :])
            bqc = cp.tile([128, G], f32, tag="bqc")
            nc.sync.dma_start(bqc[:], bq_d[:, :])
            bkc = cp.tile([128, G], f32, tag="bkc")
            nc.sync.dma_start(bkc[:], bk_d[:, :])
            bvr = cp.tile([1, HD], bf16, tag="bvr")
            nc.sync.dma_start(bvr[:], bv_d[:, :])
            ones = cp.tile([1, 128], bf16, tag="ones")
            nc.vector.memset(ones[:], 1.0)
            ones8 = cp.tile([128, HC], f32, tag="ones8")
            nc.vector.memset(ones8[:], 1.0)
            wq_s, wk_s, wv_s = [], [], []
            for dt in range(DT):
                for lst, dram, nm in ((wq_s, wq_d, "wq"), (wk_s, wk_d, "wk"),
                                      (wv_s, wv_d, "wv")):
                    t = cp.tile([128, HD], bf16, name=f"{nm}{dt}", tag=f"{nm}{dt}")
                    nc.sync.dma_start(t[:], dram[dt * 128:(dt + 1) * 128, :])
                    lst.append(t)
            xT_s = [cp.tile([128, S], bf16, name=f"xT{dt}", tag=f"xT{dt}")
                    for dt in range(DT)]
            for c in range(S // CH):
                for dt in range(DT):
                    nc.sync.dma_start(
                        xT_s[dt][:, c * CH:(c + 1) * CH],
                        xT_d[dt * 128:(dt + 1) * 128, c * CH:(c + 1) * CH])
            wo_s = []
            for wt in range(WOT):
                t = cp.tile([128, D], bf16, name=f"wo{wt}", tag=f"wo{wt}")
                nc.sync.dma_start(t[:], wo_d[wt * 128:(wt + 1) * 128, :])
                wo_s.append(t)

            qT = [cp.tile([128, S], bf16, name=f"qT{g}", tag=f"qT{g}")
                  for g in range(G)]
            kT = [cp.tile([128, S], bf16, name=f"kT{g}", tag=f"kT{g}")
                  for g in range(G)]
            vP = [cp.tile([128, HC * (dh + 1)], bf16, name=f"vP{kt}",
                          tag=f"vP{kt}") for kt in range(KT)]
            oT = [cp.tile([128, S], bf16, name=f"oT{g}", tag=f"oT{g}")
                  for g in range(G)]

            # mask columns of V' (col 64 of each head's 65-block), written
            # up-front: vP[st][:, 64::65] = mask * ones8
            for st in range(KT):
                mc = vP[st][:, :].rearrange("p (h c) -> p h c", c=dh + 1)[:, :, dh]
                nc.vector.tensor_scalar(mc, ones8[:], mkf[:, st:st + 1], None,
                                        op0=Mult)

            with tc.tile_pool(name="pp", bufs=2, space="PSUM") as pp, \
                 tc.tile_pool(name="pss", bufs=2, space="PSUM") as pss, \
                 tc.tile_pool(name="pso", bufs=1, space="PSUM") as pso, \
                 tc.tile_pool(name="ptp", bufs=3) as ptp, \
                 tc.tile_pool(name="nrm", bufs=2) as nrm:

                def v_proj(st):
                    pv = pp.tile([128, CH], f32, name="pv", tag="pp")
                    for dt in range(DT):
                        nc.tensor.matmul(
                            pv[:, :HD],
                            lhsT=xT_s[dt][:, st * 128:(st + 1) * 128],
                            rhs=wv_s[dt][:], start=(dt == 0), stop=False)
                    nc.tensor.matmul(pv[:, :HD], lhsT=ones[:], rhs=bvr[:],
                                     start=False, stop=True)
                    vdst = vP[st][:, :].rearrange(
                        "p (h c) -> p h c", c=dh + 1)[:, :, 0:dh]
                    vsrc = pv[:, :HD].rearrange(
                        "p (h c) -> p h c", c=dh)[:, :, :]
                    nc.vector.tensor_scalar(vdst, vsrc, mkf[:, st:st + 1],
                                            None, op0=Mult)

                def qk_proj(g):
                    for dst, w_s, bcol in ((qT, wq_s, bqc), (kT, wk_s, bkc)):
                        for c in range(S // CH):
                            pq = pp.tile([128, CH], f32, name="pq", tag="pp")
                            for dt in range(DT):
                                nc.tensor.matmul(
                                    pq[:],
                                    lhsT=w_s[dt][:, g * 128:(g + 1) * 128],
                                    rhs=xT_s[dt][:, c * CH:(c + 1) * CH],
                                    start=(dt == 0), stop=(dt == DT - 1))
                            nc.vector.tensor_scalar_add(
                                dst[g][:, c * CH:(c + 1) * CH],
                                pq[:], bcol[:, g:g + 1])

                def attention(h, interleave_v=False, passes=None):
                    g, off = h // 2, (h % 2) * 64
                    for p in (range(NP) if passes is None else passes):
                        q0 = p * QHW
                        op = pso.tile([65, QHW], f32, name="op", tag="op")
                        for kt in range(KT):
                            if interleave_v and p == 0:
                                v_proj(kt)
                            sp = pss.tile([128, QHW], f32, name="sp", tag="sp")
                            for c in range(SC):
                                nc.tensor.matmul(
                                    sp[:, c * CH:(c + 1) * CH],
                                    lhsT=kT[g][off:off + 64,
                                               kt * 128:(kt + 1) * 128],
                                    rhs=qT[g][off:off + 64,
                                              q0 + c * CH:q0 + (c + 1) * CH],
                                    start=True, stop=True)
                            pt = ptp.tile([128, QHW], bf16, name="pt", tag="pt")
                            if probe == "smallexp":
                                nc.scalar.activation(pt[:, :QHW // 4],
                                                     sp[:, :QHW // 4], Exp)
                            else:
                                nc.scalar.activation(pt[:], sp[:], Exp)
                            for c in range(SC):
                                if probe == "smallpv" and kt not in (0, KT - 1):
                                    continue
                                nc.tensor.matmul(
                                    op[:, c * CH:(c + 1) * CH],
                                    lhsT=vP[kt][:, h * 65:(h + 1) * 65],
                                    rhs=pt[:, c * CH:(c + 1) * CH],
                                    start=(kt == 0), stop=(kt == KT - 1))
                        rr = nrm.tile([1, QHW], f32, name="rr", tag="rr")
                        nc.vector.reciprocal(rr[:], op[64:65, :])
                        bc = nrm.tile([64, QHW], f32, name="bc", tag="bc")
                        nc.gpsimd.partition_broadcast(bc[:], rr[:])
                        nc.vector.tensor_tensor(
                            oT[g][off:off + 64, q0:q0 + QHW],
                            op[0:64, :], bc[:], op=Mult)

                def final_block(qts):
                    # output projection through the shared 1-bank pp pool
                    for qt in qts:
                        for c in range(DCH):
                            pf = pp.tile([128, CH], f32, name="pf", tag="pp")
                            for wt in range(WOT):
                                nc.tensor.matmul(
                                    pf[:],
                                    lhsT=oT[wt][:, qt * 128:(qt + 1) * 128],
                                    rhs=wo_s[wt][:, c * CH:(c + 1) * CH],
                                    start=(wt == 0), stop=(wt == WOT - 1))
                            ys = ysb.tile([128, CH], f32, name="ys", tag="ys")
                            nc.vector.tensor_copy(ys[:], pf[:])
                            nc.sync.dma_start(
                                y_d[qt * 128:(qt + 1) * 128,
                                    c * CH:(c + 1) * CH], ys[:])

                with tc.tile_pool(name="ysb", bufs=3) as ysb:
                    qk_proj(0)
                    if NP == 1:  # small configs: no 2nd pass to hide V behind
                        for st in range(KT):
                            v_proj(st)
                    for g in range(G):
                        attention(2 * g, interleave_v=(g == 0 and NP > 1))
                        if g + 1 < G:
                            qk_proj(g + 1)
                        if g + 1 < G or NP == 1:
                            attention(2 * g + 1)
                    if NP > 1:
                        # last head: overlap the first final half with pass 1
                        attention(HC - 1, passes=[0])
                        nqt = S // 128
                        final_block(range(0, nqt * (NP - 1) // NP))
                        attention(HC - 1, passes=list(range(1, NP)))
                        final_block(range(nqt * (NP - 1) // NP, nqt))
                    else:
                        final_block(range(S // 128))

    nc.compile()
    return nc


def host_shard(x, mask, Wq, bq, Wk, bk, Wv, bv, Wo, bo,
               S=S, D=D, HC=HC, dh=DH):
    """Build the 8 per-core input maps (host-side layout prep)."""
    KT = S // 128
    G = HC // 2
    HD = HC * dh
    scale = 1.0 / np.sqrt(dh)
    in_maps = []
    x = np.asarray(x, np.float32)
    mask = np.asarray(mask)
    for c in range(N_CORES):
        b = c // 2
        hs = (c % 2) * HD  # column offset into D for this core's heads
        cols = slice(hs, hs + HD)
        m = 1.0 - mask[b].astype(np.float32)          # [S], 0 = masked out
        in_maps.append({
            "xT": np.ascontiguousarray(x[b].T).astype(BF16),
            "wq": (np.asarray(Wq)[:, cols] * scale).astype(BF16),
            "wk": np.asarray(Wk)[:, cols].astype(BF16),
            "wv": np.asarray(Wv)[:, cols].astype(BF16),
            "wo": np.asarray(Wo)[cols, :].astype(BF16),
            "mkf": np.ascontiguousarray(m.reshape(KT, 128).T),
            "bqc": np.ascontiguousarray(
                (np.asarray(bq, np.float32)[cols] * scale).reshape(G, 128).T),
            "bkc": np.ascontiguousarray(
                np.asarray(bk, np.float32)[cols].reshape(G, 128).T),
            "bvr": np.asarray(bv, np.float32)[cols].reshape(1, HD).astype(BF16),
        })
    return in_maps


def host_gather(results, bo, B=B, S=S, D=D):
    out = np.empty((B, S, D), np.float32)
    bo = np.asarray(bo, np.float32)
    for b in range(B):
        out[b] = results[2 * b]["y"] + results[2 * b + 1]["y"] + bo
    return out


_NC_CACHE = None


def kernel(x, mask, Wq, bq, Wk, bk, Wv, bv, Wo, bo):
    global _NC_CACHE
    from concourse.bass_utils import run_bass_kernel_spmd
    if _NC_CACHE is None:
        _NC_CACHE = build_nc()
    in_maps = host_shard(x, mask, Wq, bq, Wk, bk, Wv, bv, Wo, bo)
    res = run_bass_kernel_spmd(_NC_CACHE, in_maps, core_ids=list(range(N_CORES)))
    return host_gather(res.results, bo)


# revision 14
# speedup vs baseline: 1.0055x; 1.0055x over previous
"""Trainium2 Bass kernel for an eager bidirectional attention block.

Reference computation (per batch b):
    q,k,v = (x @ Wq + bq), (x @ Wk + bk), (x @ Wv + bv)   split into 16 heads of 64
    scores = q @ k^T / sqrt(dh);  scores[:, masked_k] = -inf
    out = softmax(scores) @ v;    y = concat_heads(out) @ Wo + bo

Sharding (8 cores): core c owns batch b = c//2 and heads [8*(c%2), 8*(c%2)+8).
Each core computes a partial y_c = sum_{its heads} softmax(..) v_h @ Wo[h-rows, :]
(fp32, [S, D]); the host sums the two partials per batch and adds bo.
No collectives are needed.

Per-core layout strategy (all matmuls bf16, fp32 PSUM accumulate):
  - x is uploaded transposed (x^T [D, S]) so QKV projections produce
    Q^T/K^T [dh, S] directly (2 heads packed per 128-partition tile).
  - scores are computed transposed: S^T [k, q] = (K^T-tile)^T-stationary @ Q^T.
  - mask is folded into V:  V' = [V * m | m]  (65 columns per head); the PV
    matmul O^T = V'^T @ exp(S^T) then yields both the unnormalized output
    (rows 0..63) and the softmax denominator (row 64) in one accumulation.
  - 1/sqrt(dh) is folded into Wq/bq on the host.
  - normalization: reciprocal of row 64, partition-broadcast, multiply.
  - final projection contracts the stacked normalized O^T against Wo rows.

Scheduling (v2): attention runs in q-half passes so the score PSUM tile
([128,1024], 2 banks) double-buffers against the PV accumulator ([65,1024],
2 banks) and one shared [128,512] projection pool — 8 banks total. Emission
order V-proj, QK(g0), then attention pairs with the next group's QK
projections interleaved, keeping ACT (the exp engine, the critical path)
continuously fed.
"""

import numpy as np
import ml_dtypes

BF16 = ml_dtypes.bfloat16

# Full problem dims
B, S, D, H, DH = 4, 2048, 1024, 16, 64
N_CORES = 8
HC = 8  # heads per core


def build_nc(S=S, D=D, HC=HC, dh=DH, num_devices=N_CORES, reps=1, probe=None):
    from concourse import bacc
    import concourse.mybir as mybir
    from concourse.tile import TileContext

    f32 = mybir.dt.float32
    bf16 = mybir.dt.bfloat16
    Exp = mybir.ActivationFunctionType.Exp
    Mult = mybir.AluOpType.mult

    G = HC // 2           # 2-head groups
    DT = D // 128         # contraction tiles for projections
    KT = S // 128         # key-position tiles
    CH = min(512, S)      # psum chunk width (one bank)
    HD = HC * dh          # total head dim per core (512)
    WOT = HD // 128       # Wo k-tiles (= G)
    DCH = D // CH         # output chunks in final proj
    QHW = min(1024, S)    # attention q-pass width (2 banks)
    NP = S // QHW         # q passes per head
    SC = QHW // CH        # score-chunks per pass

    nc = bacc.Bacc("TRN2", target_bir_lowering=False, debug=False,
                   num_devices=num_devices)

    xT_d = nc.dram_tensor("xT", [D, S], bf16, kind="ExternalInput").ap()
    wq_d = nc.dram_tensor("wq", [D, HD], bf16, kind="ExternalInput").ap()
    wk_d = nc.dram_tensor("wk", [D, HD], bf16, kind="ExternalInput").ap()
    wv_d = nc.dram_tensor("wv", [D, HD], bf16, kind="ExternalInput").ap()
    wo_d = nc.dram_tensor("wo", [HD, D], bf16, kind="ExternalInput").ap()
    mkf_d = nc.dram_tensor("mkf", [128, KT], f32, kind="ExternalInput").ap()
    bq_d = nc.dram_tensor("bqc", [128, G], f32, kind="ExternalInput").ap()
    bk_d = nc.dram_tensor("bkc", [128, G], f32, kind="ExternalInput").ap()
    bv_d = nc.dram_tensor("bvr", [1, HD], bf16, kind="ExternalInput").ap()
    y_d = nc.dram_tensor("y", [S, D], f32, kind="ExternalOutput").ap()

    with TileContext(nc) as tc:
      for _rep in range(reps):
        with tc.tile_pool(name="const", bufs=1) as cp:
            # Small, critical-path transfers first: the mask (gates V'
            # prep), biases, and the Q/K/V weights; x^T is split into
            # column chunks so the first projection chunk only waits on
            # the leading slice of every dt-tile; Wo last (needed at the
            # very end only).
            mkf = cp.tile([128, KT], f32, tag="mkf")
            nc.sync.dma_start(mkf[:], mkf_d[:, :])
            bqc = cp.tile([128, G], f32, tag="bqc")
            nc.sync.dma_start(bqc[:], bq_d[:, :])
            bkc = cp.tile([128, G], f32, tag="bkc")
            nc.sync.dma_start(bkc[:], bk_d[:, :])
            bvr = cp.tile([1, HD], bf16, tag="bvr")
            nc.sync.dma_start(bvr[:], bv_d[:, :])
            ones = cp.tile([1, 128], bf16, tag="ones")
            nc.vector.memset(ones[:], 1.0)
            ones8 = cp.tile([128, HC], f32, tag="ones8")
            nc.vector.memset(ones8[:], 1.0)
            wq_s, wk_s, wv_s = [], [], []
            for dt in range(DT):
                for lst, dram, nm in ((wq_s, wq_d, "wq"), (wk_s, wk_d, "wk"),
                                      (wv_s, wv_d, "wv")):
                    t = cp.tile([128, HD], bf16, name=f"{nm}{dt}", tag=f"{nm}{dt}")
                    nc.sync.dma_start(t[:], dram[dt * 128:(dt + 1) * 128, :])
                    lst.append(t)
            xT_s = [cp.tile([128, S], bf16, name=f"xT{dt}", tag=f"xT{dt}")
                    for dt in range(DT)]
            for c in range(S // CH):
                for dt in range(DT):
                    nc.sync.dma_start(
                        xT_s[dt][:, c * CH:(c + 1) * CH],
                        xT_d[dt * 128:(dt + 1) * 128, c * CH:(c + 1) * CH])
            wo_s = []
            for wt in range(WOT):
                t = cp.tile([128, D], bf16, name=f"wo{wt}", tag=f"wo{wt}")
                nc.sync.dma_start(t[:], wo_d[wt * 128:(wt + 1) * 128, :])
                wo_s.append(t)

            qT = [cp.tile([128, S], bf16, name=f"qT{g}", tag=f"qT{g}")
                  for g in range(G)]
            kT = [cp.tile([128, S], bf16, name=f"kT{g}", tag=f"kT{g}")
                  for g in range(G)]
            vP = [cp.tile([128, HC * (dh + 1)], bf16, name=f"vP{kt}",
                          tag=f"vP{kt}") for kt in range(KT)]
            oT = [cp.tile([128, S], bf16, name=f"oT{g}", tag=f"oT{g}")
                  for g in range(G)]

            # mask columns of V' (col 64 of each head's 65-block), written
            # up-front: vP[st][:, 64::65] = mask * ones8
            for st in range(KT):
                mc = vP[st][:, :].rearrange("p (h c) -> p h c", c=dh + 1)[:, :, dh]
                nc.vector.tensor_scalar(mc, ones8[:], mkf[:, st:st + 1], None,
                                        op0=Mult)

            with tc.tile_pool(name="pp", bufs=2, space="PSUM") as pp, \
                 tc.tile_pool(name="pss", bufs=2, space="PSUM") as pss, \
                 tc.tile_pool(name="pso", bufs=1, space="PSUM") as pso, \
                 tc.tile_pool(name="ptp", bufs=3) as ptp, \
                 tc.tile_pool(name="nrm", bufs=2) as nrm:

                def v_proj(st):
                    pv = pp.tile([128, CH], f32, name="pv", tag="pp")
                    for dt in range(DT):
                        nc.tensor.matmul(
                            pv[:, :HD],
                            lhsT=xT_s[dt][:, st * 128:(st + 1) * 128],
                            rhs=wv_s[dt][:], start=(dt == 0), stop=False)
                    nc.tensor.matmul(pv[:, :HD], lhsT=ones[:], rhs=bvr[:],
                                     start=False, stop=True)
                    vdst = vP[st][:, :].rearrange(
                        "p (h c) -> p h c", c=dh + 1)[:, :, 0:dh]
                    vsrc = pv[:, :HD].rearrange(
                        "p (h c) -> p h c", c=dh)[:, :, :]
                    nc.vector.tensor_scalar(vdst, vsrc, mkf[:, st:st + 1],
                                            None, op0=Mult)

                def qk_proj(g):
                    for dst, w_s, bcol in ((qT, wq_s, bqc), (kT, wk_s, bkc)):
                        for c in range(S // CH):
                            pq = pp.tile([128, CH], f32, name="pq", tag="pp")
                            for dt in range(DT):
                                nc.tensor.matmul(
                                    pq[:],
                                    lhsT=w_s[dt][:, g * 128:(g + 1) * 128],
                                    rhs=xT_s[dt][:, c * CH:(c + 1) * CH],
                                    start=(dt == 0), stop=(dt == DT - 1))
                            nc.vector.tensor_scalar_add(
                                dst[g][:, c * CH:(c + 1) * CH],
                                pq[:], bcol[:, g:g + 1])

                def attention(h, interleave_v=False, passes=None):
                    g, off = h // 2, (h % 2) * 64
                    for p in (range(NP) if passes is None else passes):
                        q0 = p * QHW
                        op = pso.tile([65, QHW], f32, name="op", tag="op")
                        for kt in range(KT):
                            if interleave_v and p == 0:
                                v_proj(kt)
                            sp = pss.tile([128, QHW], f32, name="sp", tag="sp")
                            for c in range(SC):
                                nc.tensor.matmul(
                                    sp[:, c * CH:(c + 1) * CH],
                                    lhsT=kT[g][off:off + 64,
                                               kt * 128:(kt + 1) * 128],
                                    rhs=qT[g][off:off + 64,
                                              q0 + c * CH:q0 + (c + 1) * CH],
                                    start=True, stop=True)
                            pt = ptp.tile([128, QHW], bf16, name="pt", tag="pt")
                            if probe == "smallexp":
                                nc.scalar.activation(pt[:, :QHW // 4],
                                                     sp[:, :QHW // 4], Exp)
                            else:
                                nc.scalar.activation(pt[:], sp[:], Exp)
                            for c in range(SC):
                                if probe == "smallpv" and kt not in (0, KT - 1):
                                    continue
                                nc.tensor.matmul(
                                    op[:, c * CH:(c + 1) * CH],
                                    lhsT=vP[kt][:, h * 65:(h + 1) * 65],
                                    rhs=pt[:, c * CH:(c + 1) * CH],
                                    start=(kt == 0), stop=(kt == KT - 1))
                        # stage to SBUF with one copy so the PSUM slot frees
                        # immediately; normalize off the critical path
                        stg = nrm.tile([65, QHW], f32, name="stg", tag="stg")
                        nc.vector.tensor_copy(stg[:], op[:])
                        rr = nrm.tile([1, QHW], f32, name="rr", tag="rr")
                        nc.vector.reciprocal(rr[:], stg[64:65, :])
                        bc = nrm.tile([64, QHW], f32, name="bc", tag="bc")
                        nc.gpsimd.partition_broadcast(bc[:], rr[:])
                        nc.vector.tensor_tensor(
                            oT[g][off:off + 64, q0:q0 + QHW],
                            stg[0:64, :], bc[:], op=Mult)

                def final_block(qts):
                    # output projection through the shared 1-bank pp pool
                    for qt in qts:
                        for c in range(DCH):
                            pf = pp.tile([128, CH], f32, name="pf", tag="pp")
                            for wt in range(WOT):
                                nc.tensor.matmul(
                                    pf[:],
                                    lhsT=oT[wt][:, qt * 128:(qt + 1) * 128],
                                    rhs=wo_s[wt][:, c * CH:(c + 1) * CH],
                                    start=(wt == 0), stop=(wt == WOT - 1))
                            ys = ysb.tile([128, CH], f32, name="ys", tag="ys")
                            nc.vector.tensor_copy(ys[:], pf[:])
                            nc.sync.dma_start(
                                y_d[qt * 128:(qt + 1) * 128,
                                    c * CH:(c + 1) * CH], ys[:])

                with tc.tile_pool(name="ysb", bufs=3) as ysb:
                    qk_proj(0)
                    if NP == 1:  # small configs: no 2nd pass to hide V behind
                        for st in range(KT):
                            v_proj(st)
                    for g in range(G):
                        attention(2 * g, interleave_v=(g == 0 and NP > 1))
                        if g + 1 < G:
                            qk_proj(g + 1)
                        if g + 1 < G or NP == 1:
                            attention(2 * g + 1)
                    if NP > 1:
                        # last head: overlap the first final half with pass 1
                        attention(HC - 1, passes=[0])
                        nqt = S // 128
                        final_block(range(0, nqt * (NP - 1) // NP))
                        attention(HC - 1, passes=list(range(1, NP)))
                        final_block(range(nqt * (NP - 1) // NP, nqt))
                    else:
                        final_block(range(S // 128))

    nc.compile()
    return nc


def host_shard(x, mask, Wq, bq, Wk, bk, Wv, bv, Wo, bo,
               S=S, D=D, HC=HC, dh=DH):
    """Build the 8 per-core input maps (host-side layout prep)."""
    KT = S // 128
    G = HC // 2
    HD = HC * dh
    scale = 1.0 / np.sqrt(dh)
    in_maps = []
    x = np.asarray(x, np.float32)
    mask = np.asarray(mask)
    for c in range(N_CORES):
        b = c // 2
        hs = (c % 2) * HD  # column offset into D for this core's heads
        cols = slice(hs, hs + HD)
        m = 1.0 - mask[b].astype(np.float32)          # [S], 0 = masked out
        in_maps.append({
            "xT": np.ascontiguousarray(x[b].T).astype(BF16),
            "wq": (np.asarray(Wq)[:, cols] * scale).astype(BF16),
            "wk": np.asarray(Wk)[:, cols].astype(BF16),
            "wv": np.asarray(Wv)[:, cols].astype(BF16),
            "wo": np.asarray(Wo)[cols, :].astype(BF16),
            "mkf": np.ascontiguousarray(m.reshape(KT, 128).T),
            "bqc": np.ascontiguousarray(
                (np.asarray(bq, np.float32)[cols] * scale).reshape(G, 128).T),
            "bkc": np.ascontiguousarray(
                np.asarray(bk, np.float32)[cols].reshape(G, 128).T),
            "bvr": np.asarray(bv, np.float32)[cols].reshape(1, HD).astype(BF16),
        })
    return in_maps


def host_gather(results, bo, B=B, S=S, D=D):
    out = np.empty((B, S, D), np.float32)
    bo = np.asarray(bo, np.float32)
    for b in range(B):
        out[b] = results[2 * b]["y"] + results[2 * b + 1]["y"] + bo
    return out


_NC_CACHE = None


def kernel(x, mask, Wq, bq, Wk, bk, Wv, bv, Wo, bo):
    global _NC_CACHE
    from concourse.bass_utils import run_bass_kernel_spmd
    if _NC_CACHE is None:
        _NC_CACHE = build_nc()
    in_maps = host_shard(x, mask, Wq, bq, Wk, bk, Wv, bv, Wo, bo)
    res = run_bass_kernel_spmd(_NC_CACHE, in_maps, core_ids=list(range(N_CORES)))
    return host_gather(res.results, bo)
